# revision 35
# baseline (speedup 1.0000x reference)
"""DeepFM dense-MLP kernel for 8x Trainium2 NeuronCores (Bass/Tile).

Computation (reference):
    inter = relu(x * x.sum(axis=1, keepdims=True))        # FM pairwise term
    h = x
    for i in 0..3:  h = relu(h @ W_dnn[i].T + b_dnn[i])
    out = ((h + inter) * 0.5) @ W_out.T + b_out

Strategy:
  - Data-parallel: batch B=8192 split across 8 cores (1024 rows each).
  - Feature-major activations on device: h^T [D, B_c] so every GEMM is
    psum[e, b] += W^T.T @ h^T with the weight tile stationary.
  - Precision split: the output norm is dominated by the FM term
    (inter ~ 45 rms vs h4 ~ 1), so the 4 hidden layers contribute only
    ~1.5% of the output. They run in fp8 e4m3 with DoubleRow perf mode:
    each PE cell holds 2 weights, lhsT [128, 2, 128] -> K=256, M=128
    per 512-cycle pass = 2x bf16 MAC rate (157 TF/s, measured 216ns
    per matmul, same as bf16's [128,128]x[128,512]).
    Only the final GEMM runs in bf16. End-to-end rel err ~4e-3, same
    as all-bf16.
  - fp8 weights are pre-scaled x64 on host (sigma_W=0.02 is below the
    e4m3 normal range 2^-6); the eviction ACT op computes
    relu(psum * 2^-6 + bias) in one fused pass.
  - Row-sum s = sum_d x[b, d] is computed on host (pure input prep)
    and shipped as bf16 0.5*s; broadcast across partitions via a K=1
    ones matmul. The FM term is DVE-built from a streamed bf16 copy of
    x (fp8 x would cost 2.6% error) in place into the bf16 z buffer,
    overlapping layer-3 compute. The 0.5 scale on the last-layer input
    is folded into layer-3's weights and bias.
"""

import sys

import ml_dtypes
import numpy as np

if "/opt/trn_rl_repo" not in sys.path:
    sys.path.insert(0, "/opt/trn_rl_repo")

import concourse.mybir as mybir  # noqa: E402
import concourse.tile as tile  # noqa: E402
from concourse import bacc  # noqa: E402
from concourse.bass_utils import run_bass_kernel_spmd  # noqa: E402

B, D, L = 8192, 4096, 4
NCORES = 8
BC = B // NCORES  # 1024 batch rows per core
P = 128
KT = D // 256  # 16 k-blocks of 256 (fp8 DoubleRow)
KK = D // P  # 32 k-tiles of 128 (bf16 layer)
MM = D // P  # 32 m-tiles of 128
NB = 512  # matmul free dim / PSUM bank
PI = BC // NB  # inner passes
NLAYERS = 5
WSCALE = 64.0  # fp8 weight pre-scale (undone at eviction)

f32 = mybir.dt.float32
bf16 = mybir.dt.bfloat16
fp8 = mybir.dt.float8e4

NP_BF16 = ml_dtypes.bfloat16
NP_FP8 = mybir.dt.np(fp8)


def _build():
    nc = bacc.Bacc(None, target_bir_lowering=False, debug=False)
    x8_p = nc.declare_dram_parameter("x8", [KT, P, 2 * BC], fp8, isOutput=False)
    xb_p = nc.declare_dram_parameter("xb", [KK, P, BC], bf16, isOutput=False)
    sb_p = nc.declare_dram_parameter("sb", [1, BC], bf16, isOutput=False)
    w8_p = nc.declare_dram_parameter("w8", [L, MM, P, KT * 256], fp8, isOutput=False)
    w5_p = nc.declare_dram_parameter("w5", [MM, P, KK * P], bf16, isOutput=False)
    bias_p = nc.declare_dram_parameter("bias", [NLAYERS, P, MM], f32, isOutput=False)
    out_p = nc.declare_dram_parameter("out", [MM, P, BC], f32, isOutput=True)

    add = mybir.AluOpType.add
    amax = mybir.AluOpType.max
    dr = mybir.MatmulPerfMode.DoubleRow

    with tile.TileContext(nc) as tc:
        with (
            tc.tile_pool(name="const", bufs=1) as const,
            tc.tile_pool(name="hA", bufs=1) as hA_pool,
            tc.tile_pool(name="hB", bufs=1) as hB_pool,
            tc.tile_pool(name="zb", bufs=1) as z_pool,
            tc.tile_pool(name="w8s", bufs=5) as w8pool,
            tc.tile_pool(name="w5s", bufs=2) as w5pool,
            tc.tile_pool(name="xst", bufs=3) as xpool,
            tc.tile_pool(name="tmp", bufs=3) as tpool,
            tc.tile_pool(name="outt", bufs=3) as opool,
            tc.tile_pool(name="sbb", bufs=1) as sbbpool,
            tc.tile_pool(name="sBt", bufs=2) as sBpool,
            tc.tile_pool(name="psum", bufs=8, space="PSUM") as psum_pool,
        ):
            ones_1p = const.tile([1, P], bf16)
            nc.any.memset(ones_1p[:], 1.0)

            # first weight strip and the tiny sb vector go out before
            # everything else so the PE can start as soon as possible
            wt0 = w8pool.tile([P, KT, 2, P], fp8, name="w8t")
            nc.scalar.dma_start(out=wt0[:], in_=w8_p[0, 0])
            s_sb = sbbpool.tile([1, BC], bf16, name="s_sb")
            nc.sync.dma_start(out=s_sb[:], in_=sb_p[:])

            # fp8 activation ping-pong buffers; FA starts as x8
            FA = [hA_pool.tile([P, 2, BC], fp8, name=f"hA{k}") for k in range(KT)]
            FB = [hB_pool.tile([P, 2, BC], fp8, name=f"hB{k}") for k in range(KT)]
            Z = [z_pool.tile([P, BC], bf16, name=f"z{k}") for k in range(KK)]
            # split the initial loads across both HW DMA queues (SP + ACT)
            for kt in range(KT):
                eng = nc.sync if kt % 2 == 0 else nc.scalar
                eng.dma_start(out=FA[kt][:], in_=x8_p[kt])

            # bias is first needed at layer 0's first eviction, well after
            # the x8 stream; keep it off the critical path
            bias_t = const.tile([P, NLAYERS * MM], f32)
            for l in range(NLAYERS):
                nc.sync.dma_start(out=bias_t[:, l * MM : (l + 1) * MM], in_=bias_p[l])

            # warm the PE during the x8 DMA window: the tensor engine
            # ramps to full clock only after ~3us of continuous work, so
            # burn the ramp on dummy full-length matmuls (K=1) and the sB
            # broadcast instead of on layer 0's real groups
            sB = []
            ps_w = psum_pool.tile([P, NB], f32, name="ps")
            for i in range(12):
                nc.tensor.matmul(
                    ps_w[:], ones_1p[:], s_sb[:, 0:NB],
                    start=(i == 0), stop=(i == 11),
                )
            warm_sink = tpool.tile([P, NB], f32, name="tmp")
            nc.vector.tensor_copy(out=warm_sink[:], in_=ps_w[:])
            for pi in range(PI):
                ps_b = psum_pool.tile([P, NB], f32, name="ps")
                nc.tensor.matmul(
                    ps_b[:],
                    ones_1p[:],
                    s_sb[:, pi * NB : (pi + 1) * NB],
                    start=True,
                    stop=True,
                )
                sBt = sBpool.tile([P, NB], f32, name=f"sB{pi}")
                nc.vector.tensor_copy(out=sBt[:], in_=ps_b[:])
                sB.append(sBt)

            # hidden layers in fp8 DoubleRow: FA->FB->FA->FB->Z(bf16)
            for l in range(L):
                src = FA if l % 2 == 0 else FB
                dst = FB if l % 2 == 0 else FA
                for m in range(MM):
                    if l == 0 and m == 0:
                        wt = wt0
                    else:
                        wt = w8pool.tile([P, KT, 2, P], fp8, name="w8t")
                        weng = nc.scalar if m % 2 == 0 else nc.sync
                        weng.dma_start(out=wt[:], in_=w8_p[l, m])
                    for pi in range(PI):
                        csl = slice(pi * NB, (pi + 1) * NB)
                        ps = psum_pool.tile([P, NB], f32, name="ps")
                        for kt in range(KT):
                            nc.tensor.matmul(
                                ps[:],
                                wt[:, kt],
                                src[kt][:, :, csl],
                                start=(kt == 0),
                                stop=(kt == KT - 1),
                                perf_mode=dr,
                            )
                        bsl = bias_t[:, l * MM + m : l * MM + m + 1]
                        if l < L - 1:
                            dap = dst[m // 2][:, m % 2, csl]
                        else:
                            dap = Z[m][:, csl]
                        nc.scalar.activation(
                            dap,
                            ps[:],
                            mybir.ActivationFunctionType.Relu,
                            bias=bsl,
                            scale=1.0 / WSCALE,
                        )
            # Z[kk] += relu(x^T * 0.5 s)   (h5in build, in place)
            for kk in range(KK):
                xt = xpool.tile([P, BC], bf16, name="xs")
                nc.sync.dma_start(out=xt[:], in_=xb_p[kk])
                for pi in range(PI):
                    csl = slice(pi * NB, (pi + 1) * NB)
                    tmp = tpool.tile([P, NB], f32, name="tmp")
                    nc.vector.tensor_mul(out=tmp[:], in0=xt[:, csl], in1=sB[pi][:])
                    nc.vector.scalar_tensor_tensor(
                        out=Z[kk][:, csl],
                        in0=tmp[:],
                        scalar=0.0,
                        in1=Z[kk][:, csl],
                        op0=amax,
                        op1=add,
                    )

            # output layer in bf16
            lo = NLAYERS - 1
            for m in range(MM):
                wt = w5pool.tile([P, KK * P], bf16, name="w5t")
                weng = nc.scalar if m % 2 == 0 else nc.sync
                weng.dma_start(out=wt[:], in_=w5_p[m])
                for pi in range(PI):
                    csl = slice(pi * NB, (pi + 1) * NB)
                    ps = psum_pool.tile([P, NB], f32, name="ps")
                    for kk in range(KK):
                        nc.tensor.matmul(
                            ps[:],
                            wt[:, kk * P : (kk + 1) * P],
                            Z[kk][:, csl],
                            start=(kk == 0),
                            stop=(kk == KK - 1),
                        )
                    bsl = bias_t[:, lo * MM + m : lo * MM + m + 1]
                    ot = opool.tile([P, NB], f32, name="ot")
                    nc.vector.tensor_scalar_add(out=ot[:], in0=ps[:], scalar1=bsl)
                    oeng = nc.scalar if m % 2 == 0 else nc.sync
                    oeng.dma_start(
                        out=out_p[m][:, pi * NB : (pi + 1) * NB], in_=ot[:]
                    )
    nc.compile()
    return nc


_NC_CACHE = {}


def _get_nc():
    if "nc" not in _NC_CACHE:
        _NC_CACHE["nc"] = _build()
    return _NC_CACHE["nc"]


def _prep_weights(W_dnn, W_out, b_dnn, b_out):
    # fp8 hidden weights, x64 pre-scale, DoubleRow layout:
    # w8[l, m, p, kt, i, mc] = 64*W_l[m*128+mc, kt*256+i*128+p]
    w8 = np.empty((L, MM, P, KT * 256), dtype=NP_FP8)
    for l in range(L):
        Wl = np.asarray(W_dnn[l], dtype=np.float32)
        if l == L - 1:
            Wl = Wl * 0.5  # fold the (h+inter)*0.5 into layer 3's output
        q = (Wl * WSCALE).astype(NP_FP8)
        # [E, Din] -> [m, mc, kt, i, p] -> [m, p, kt, i, mc]
        w8[l] = (
            q.reshape(MM, P, KT, 2, P)
            .transpose(0, 4, 2, 3, 1)
            .reshape(MM, P, KT * 256)
        )
    W5 = np.asarray(W_out, dtype=np.float32)
    w5 = (
        W5.reshape(MM, P, KK, P)
        .transpose(0, 3, 2, 1)
        .reshape(MM, P, KK * P)
        .astype(NP_BF16)
    )
    b_all = np.empty((NLAYERS, P, MM), dtype=np.float32)
    for l in range(NLAYERS):
        bl = np.asarray(b_dnn[l] if l < L else b_out, dtype=np.float32)
        if l == L - 1:
            bl = bl * 0.5
        b_all[l] = bl.reshape(MM, P).T
    return w8, w5, b_all


def _prep_x(xc):
    # xc [BC, D] -> x8 [KT, P, 2*BC] fp8 (two 128-row k-slices per block)
    #            -> xb [KK, P, BC] bf16
    xT = np.ascontiguousarray(xc.T)  # [D, BC]
    x8 = (
        xT.reshape(KT, 2, P, BC)
        .transpose(0, 2, 1, 3)
        .reshape(KT, P, 2 * BC)
        .astype(NP_FP8)
    )
    xb = xT.reshape(KK, P, BC).astype(NP_BF16)
    sb = (0.5 * xc.sum(axis=1, dtype=np.float64)).astype(NP_BF16).reshape(1, BC)
    return x8, xb, sb


def _make_in_maps(x, W_dnn, b_dnn, W_out, b_out):
    x = np.asarray(x, dtype=np.float32)
    w8, w5, b_all = _prep_weights(W_dnn, W_out, b_dnn, b_out)
    in_maps = []
    for c in range(NCORES):
        x8, xb, sb = _prep_x(x[c * BC : (c + 1) * BC])
        in_maps.append(
            {"x8": x8, "xb": xb, "sb": sb, "w8": w8, "w5": w5, "bias": b_all}
        )
    return in_maps


def kernel(x, W_dnn, b_dnn, W_out, b_out):
    in_maps = _make_in_maps(x, W_dnn, b_dnn, W_out, b_out)
    nc = _get_nc()
    res = run_bass_kernel_spmd(nc, in_maps, list(range(NCORES)))
    out = np.empty((B, D), dtype=np.float32)
    for c in range(NCORES):
        out[c * BC : (c + 1) * BC] = res.results[c]["out"].reshape(D, BC).T
    return out
